# revision 6
# baseline (speedup 1.0000x reference)
"""Binarized ResNet BasicBlock (2x binarized 3x3 conv + batchnorm + hardtanh,
residual) on 8 Trainium2 NeuronCores, data-parallel over batch.

Math (per reference):
  s1  = conv3x3(sign(x), sign(W1), pad=1)          # integer-valued
  h   = clip(bn1(s1), -1, 1)                       # only sign(h) is consumed
  s2p = conv3x3(sign(h), sign(W2), pad=1) + x
  out = clip(bn2(s2p), -1, 1)

Key points:
  - sign(h) = sign(a1*s1 + c1) per channel (a1 = g1*rsqrt(v1+eps),
    c1 = b1 - m1*a1, and sign(g*z) = sign(z)*sign(g) handled by the affine
    form directly), so h is never materialized.
  - batchnorm uses global batch stats: each core computes per-channel
    (E[x], E[x^2]) partials over its 4 images; a tiny AllReduce (128x6 f32)
    combines them (all cores hold equal pixel counts, so mean-of-means works).
  - conv is 27 accumulated 128x128 matmuls per output tile (3 input-channel
    chunks x 9 taps) in bf16; +/-1 inputs make the fp32 PSUM accumulation
    exact.
  - s1 and s2p stay resident in SBUF as fp16 (integer magnitudes < 2048:
    exact; s2p adds the fp32 residual, fp16 rounding ~5e-4 relative).
"""

import numpy as np
import ml_dtypes

import concourse.bass as bass
import concourse.tile as tile
from concourse import bacc, mybir
from concourse.bass_utils import run_bass_kernel_spmd
from concourse.replica_groups import maybe_share_collective_output_space

F32 = mybir.dt.float32
F16 = mybir.dt.float16
BF16 = mybir.dt.bfloat16

NCORES = 8
B, C, H, W = 32, 384, 56, 56
P = C
BPC = B // NCORES         # images per core
NCC = C // 128            # input channel chunks
NPC = P // 128            # output channel chunks
HP = H + 2                # padded rows
WP = W + 2                # padded cols
PADPIX = HP * WP
NPIX = H * W
CHUNK_ROWS = 8            # output rows per PSUM tile
NCHUNK = H // CHUNK_ROWS  # 7
CHW = CHUNK_ROWS * W      # 448 <= 512 (one PSUM bank)
COUNT = B * NPIX          # global batchnorm pixel count per channel
EPS = 1e-5

MODE = "fp8"              # "bf16" or "fp8"
F8 = mybir.dt.float8e4
F8NP = mybir.dt.np(F8)
CSTRIDE = 3376            # fp8: per-cc padded plane stride (16B-aligned)
RUN = CHUNK_ROWS * WP     # fp8: contiguous rhs run incl. seam (464 <= 512)

# fp8 DoubleRow unit schedule: 12 pairs (K=256) + 3 singles (K=128).
# Copy "A" is sign(plane) at shift 0, copy "B" at shift +1 col (so dx=1
# taps read from an even base address, per the 2B-alignment rule).
# pair type "dx01": (cc, dy, dx=0)@A with (cc, dy, dx=1)@B -> rhs
#   sx[:, :, cc, pix0:pix0+RUN], pix0 = (y0+dy)*WP
# pair type "cc01": (cc0, dy, dx=2)@A with (cc1, dy, dx=2)@A -> rhs
#   sx[:, 0, 0:2, pix0:pix0+RUN], pix0 = (y0+dy)*WP + 2
# single: (cc2, dy, dx=2)@A -> rhs sx[:, 0, 2, pix0:pix0+RUN]
FP8_PAIRS = (
    [("dx01", cc, dy) for dy in range(3) for cc in range(3)]
    + [("cc01", None, dy) for dy in range(3)]
)
FP8_SINGLES = [dy for dy in range(3)]
NUNIT_FP8 = len(FP8_PAIRS) + len(FP8_SINGLES)  # 15


def _prep_weight(w):
    """[P, C, 3, 3] float32 -> [NCC, 128, 9*NPC*128] bf16 sign, laid out so
    lhsT for (cc, off, pc) is wsb[cc][:, (off*NPC+pc)*128 : +128]."""
    ws = np.sign(w.astype(np.float32)).astype(ml_dtypes.bfloat16)
    # [P, C, 3, 3] -> [C, 9, P] -> [NCC, 128, 9, NPC, 128]
    arr = ws.transpose(1, 2, 3, 0).reshape(C, 9, P)
    arr = arr.reshape(NCC, 128, 9, NPC, 128)
    return np.ascontiguousarray(arr.reshape(NCC, 128, 9 * NPC * 128))


def _prep_weight_fp8(w):
    """[P, C, 3, 3] -> (pairs [128, 12*NPC*2*128], singles [128, 3*NPC*128])
    fp8e4 sign values matching the FP8_PAIRS / FP8_SINGLES schedule."""
    ws = np.sign(w.astype(np.float32))
    # arr[c_in_chunk, cc, dy, dx, pc, p] view
    arr = ws.transpose(1, 2, 3, 0).reshape(NCC, 128, 3, 3, NPC, 128)

    def unit(cc, dy, dx):  # -> [128 (c), NPC, 128 (p)]
        return arr[cc, :, dy, dx]

    wp = np.zeros((128, len(FP8_PAIRS), NPC, 2, 128), np.float32)
    for j, (kind, cc, dy) in enumerate(FP8_PAIRS):
        if kind == "dx01":
            wp[:, j, :, 0] = unit(cc, dy, 0)
            wp[:, j, :, 1] = unit(cc, dy, 1)
        else:  # cc01
            wp[:, j, :, 0] = unit(0, dy, 2)
            wp[:, j, :, 1] = unit(1, dy, 2)
    wsg = np.zeros((128, len(FP8_SINGLES), NPC, 128), np.float32)
    for k, dy in enumerate(FP8_SINGLES):
        wsg[:, k] = unit(2, dy, 2)
    return (
        np.ascontiguousarray(wp.reshape(128, -1)).astype(F8NP),
        np.ascontiguousarray(wsg.reshape(128, -1)).astype(F8NP),
    )


def _prep_vecs(g1, b1, g2, b2):
    """-> [128, NPC, 4] f32: per-partition (p_in) per-chunk (pc) gamma/beta."""
    out = np.empty((128, NPC, 4), np.float32)
    for k, v in enumerate((g1, b1, g2, b2)):
        out[:, :, k] = v.astype(np.float32).reshape(NPC, 128).T
    return out


def _stats_to_scale_bias(nc, pools, allout, vecs_sb, eps_tile, gk, bk, name,
                         ncores):
    """allout [128, NPC, 2] summed (E, E2) over cores -> a, c [128, NPC, 1]."""
    singles = pools
    Eg = singles.tile([128, NPC, 1], F32, name=f"{name}_Eg")
    E2g = singles.tile([128, NPC, 1], F32, name=f"{name}_E2g")
    var = singles.tile([128, NPC, 1], F32, name=f"{name}_var")
    tmp = singles.tile([128, NPC, 1], F32, name=f"{name}_tmp")
    sd = singles.tile([128, NPC, 1], F32, name=f"{name}_sd")
    rs = singles.tile([128, NPC, 1], F32, name=f"{name}_rs")
    a = singles.tile([128, NPC, 1], F32, name=f"{name}_a")
    c = singles.tile([128, NPC, 1], F32, name=f"{name}_c")
    nc.scalar.mul(Eg[:], allout[:, :, 0:1], 1.0 / ncores)
    nc.scalar.mul(E2g[:], allout[:, :, 1:2], 1.0 / ncores)
    nc.vector.tensor_mul(tmp[:], Eg[:], Eg[:])
    nc.vector.tensor_tensor(
        out=var[:], in0=E2g[:], in1=tmp[:], op=mybir.AluOpType.subtract
    )
    nc.scalar.activation(
        sd[:], var[:], mybir.ActivationFunctionType.Sqrt, bias=eps_tile[:], scale=1.0
    )
    nc.vector.reciprocal(out=rs[:], in_=sd[:])
    nc.vector.tensor_mul(a[:], rs[:], vecs_sb[:, :, gk : gk + 1])
    nc.vector.tensor_mul(tmp[:], Eg[:], a[:])
    nc.vector.tensor_tensor(
        out=c[:], in0=vecs_sb[:, :, bk : bk + 1], in1=tmp[:],
        op=mybir.AluOpType.subtract,
    )
    return a, c


def _emit_conv(nc, psum_pool, wsb, sx_tile, pc, chunk):
    """27 accumulated matmuls -> psum tile [128, CHW] for (pc, chunk)."""
    ps = psum_pool.tile([128, CHW], F32, name="ps", tag="ps")
    y0 = chunk * CHUNK_ROWS
    k = 0
    for cc in range(NCC):
        sx3 = sx_tile[:, cc, :].rearrange("p (h w) -> p h w", w=WP)
        for off in range(9):
            dy, dx = off // 3, off % 3
            lhsT = wsb[cc][:, (off * NPC + pc) * 128 : (off * NPC + pc + 1) * 128]
            rhs = sx3[:, y0 + dy : y0 + dy + CHUNK_ROWS, dx : dx + W]
            nc.tensor.matmul(
                ps[:], lhsT, rhs, start=(k == 0), stop=(k == 9 * NCC - 1)
            )
            k += 1
    return ps


def _emit_conv_fp8(nc, psum_pool, wp_view, ws_view, sx_tile, pc):
    """Weight-stationary fp8 DoubleRow conv for one (img, pc): returns the
    NCHUNK psum tiles [128, RUN] (valid cols = view[:, :, 0:56] of the
    (8, 58) reshape)."""
    pss = [
        psum_pool.tile([128, RUN], F32, name="ps", tag="ps")
        for _ in range(NCHUNK)
    ]
    u = 0
    for j, (kind, cc, dy) in enumerate(FP8_PAIRS):
        lhsT = wp_view[:, j, pc]
        for chunk in range(NCHUNK):
            y0 = chunk * CHUNK_ROWS
            if kind == "dx01":
                pix0 = (y0 + dy) * WP
                rhs = sx_tile[:, :, cc, pix0 : pix0 + RUN]
            else:
                pix0 = (y0 + dy) * WP + 2
                rhs = sx_tile[:, 0, 0:2, pix0 : pix0 + RUN]
            nc.tensor.matmul(
                pss[chunk][:], lhsT, rhs,
                start=(u == 0), stop=(u == NUNIT_FP8 - 1),
                perf_mode=mybir.MatmulPerfMode.DoubleRow,
            )
        u += 1
    for k, dy in enumerate(FP8_SINGLES):
        lhsT = ws_view[:, k, pc]
        for chunk in range(NCHUNK):
            y0 = chunk * CHUNK_ROWS
            pix0 = (y0 + dy) * WP + 2
            rhs = sx_tile[:, 0, 2, pix0 : pix0 + RUN]
            nc.tensor.matmul(
                pss[chunk][:], lhsT, rhs,
                start=(u == 0), stop=(u == NUNIT_FP8 - 1),
            )
        u += 1
    return pss


def build_program(bpc=BPC, ncores=NCORES, mode=MODE):
    nc = bacc.Bacc(
        "TRN2",
        target_bir_lowering=False,
        debug=False,
        enable_asserts=True,
        num_devices=ncores,
    )
    x_d = nc.dram_tensor("x", [bpc, C, H, W], F32, kind="ExternalInput").ap()
    w1_d = nc.dram_tensor("w1", [NCC, 128, 9 * NPC * 128], BF16,
                          kind="ExternalInput").ap()
    w2_d = nc.dram_tensor("w2", [NCC, 128, 9 * NPC * 128], BF16,
                          kind="ExternalInput").ap()
    vecs_d = nc.dram_tensor("vecs", [128, NPC, 4], F32, kind="ExternalInput").ap()
    out_d = nc.dram_tensor("out", [bpc, C, H, W], F32, kind="ExternalOutput").ap()

    count_pc = bpc * NPIX  # per-core per-channel pixel count

    with tile.TileContext(nc) as tc:
        with (
            tc.tile_pool(name="weights", bufs=NCC) as wpool,
            tc.tile_pool(name="singles", bufs=1) as singles,
            tc.tile_pool(name="sx", bufs=1) as sxpool,
            tc.tile_pool(name="acc", bufs=3 * bpc) as accpool,
            tc.tile_pool(name="xin", bufs=6) as xinpool,
            tc.tile_pool(name="xr", bufs=6) as xrpool,
            tc.tile_pool(name="oc", bufs=6) as ocpool,
            tc.tile_pool(name="stats", bufs=1) as stpool,
            tc.tile_pool(name="psum", bufs=8, space="PSUM") as psum_pool,
            tc.tile_pool(name="dram", bufs=1, space="DRAM") as dram,
        ):
            # ---- load constants ----
            # W1/W2 share 3 slots (tag "w"): W2 tiles allocate after the
            # last conv1 matmul releases W1, overlapping the bn1 AllReduce.
            wsb1 = []
            for cc in range(NCC):
                t1 = wpool.tile([128, 9 * NPC * 128], BF16, name=f"w1sb{cc}",
                                tag="w")
                nc.sync.dma_start(out=t1, in_=w1_d[cc])
                wsb1.append(t1)
            vecs_sb = singles.tile([128, NPC, 4], F32)
            nc.sync.dma_start(out=vecs_sb, in_=vecs_d)
            eps_tile = singles.tile([128, 1], F32)
            nc.vector.memset(eps_tile, EPS)

            # persistent padded sign buffers (border stays zero forever)
            sxt = []
            for s in range(2):
                t = sxpool.tile([128, NCC, PADPIX], BF16, name=f"sx{s}")
                nc.gpsimd.memset(t[:], 0.0)
                sxt.append(t)

            # bn1 partial stats: per pc, one bn_stats entry per (img, chunk)
            bnst1 = [
                stpool.tile([128, bpc * NCHUNK, 6], F32, name=f"bnst1_{pc}")
                for pc in range(NPC)
            ]
            bnst2 = [
                stpool.tile([128, bpc * NCHUNK, 6], F32, name=f"bnst2_{pc}")
                for pc in range(NPC)
            ]

            # ---- pass A: conv1, stats, s1 resident in fp16 ----
            s1 = {}
            for img in range(bpc):
                sx_tile = sxt[img % 2]
                for cc in range(NCC):
                    dst = sx_tile[:, cc, :].rearrange("p (h w) -> p h w", w=WP)
                    for chunk in range(NCHUNK):
                        y0 = chunk * CHUNK_ROWS
                        xin = xinpool.tile([128, CHUNK_ROWS, W], F32,
                                           name="xin", tag="xin")
                        nc.sync.dma_start(
                            out=xin,
                            in_=x_d[img, cc * 128 : (cc + 1) * 128,
                                    y0 : y0 + CHUNK_ROWS],
                        )
                        nc.scalar.activation(
                            dst[:, 1 + y0 : 1 + y0 + CHUNK_ROWS, 1 : 1 + W],
                            xin, mybir.ActivationFunctionType.Sign,
                        )
                for pc in range(NPC):
                    s1t = accpool.tile([128, NPIX], F16, name=f"s1_{img}_{pc}",
                                       tag="acc")
                    s1[(img, pc)] = s1t
                    for chunk in range(NCHUNK):
                        ps = _emit_conv(nc, psum_pool, wsb1, sx_tile, pc, chunk)
                        sl = slice(chunk * CHW, (chunk + 1) * CHW)
                        nc.scalar.copy(s1t[:, sl], ps[:])
                        nc.vector.bn_stats(
                            out=bnst1[pc][:, img * NCHUNK + chunk, :], in_=ps[:]
                        )

            # ---- bn1 stats -> AllReduce -> thresholds ----
            allin1 = singles.tile([128, NPC, 2], F32)
            for pc in range(NPC):
                mv = stpool.tile([128, 2], F32, name=f"mv1_{pc}")
                nc.vector.bn_aggr(out=mv, in_=bnst1[pc])
                nc.vector.tensor_copy(allin1[:, pc, 0:1], mv[:, 0:1])
                sq = stpool.tile([128, 1], F32, name=f"sq1_{pc}")
                nc.vector.tensor_mul(sq, mv[:, 0:1], mv[:, 0:1])
                nc.vector.tensor_tensor(
                    out=allin1[:, pc, 1:2], in0=mv[:, 1:2], in1=sq,
                    op=mybir.AluOpType.add,
                )
            cc_addr_space = maybe_share_collective_output_space(
                "AllReduce", [list(range(ncores))]
            )
            cc1_in = dram.tile([128, NPC * 2], F32, name="cc1_in")
            cc1_out = dram.tile([128, NPC * 2], F32, name="cc1_out",
                                addr_space=cc_addr_space)
            nc.gpsimd.dma_start(out=cc1_in, in_=allin1.rearrange("p a b -> p (a b)"))
            nc.gpsimd.collective_compute(
                "AllReduce",
                mybir.AluOpType.add,
                replica_groups=[list(range(ncores))],
                ins=[cc1_in.opt()],
                outs=[cc1_out.opt()],
            )
            allout1 = singles.tile([128, NPC, 2], F32)
            nc.gpsimd.dma_start(out=allout1.rearrange("p a b -> p (a b)"),
                                in_=cc1_out)
            a1, c1 = _stats_to_scale_bias(
                nc, singles, allout1, vecs_sb, eps_tile, 0, 1, "bn1", ncores
            )

            # ---- load W2 into the shared weight slots ----
            wsb2 = []
            for cc in range(NCC):
                t2 = wpool.tile([128, 9 * NPC * 128], BF16, name=f"w2sb{cc}",
                                tag="w")
                nc.sync.dma_start(out=t2, in_=w2_d[cc])
                wsb2.append(t2)

            # ---- pass B: sign threshold, conv2 + residual, stats ----
            s2 = {}
            for img in range(bpc):
                sh_tile = sxt[img % 2]
                for pc in range(NPC):
                    dst = sh_tile[:, pc, :].rearrange("p (h w) -> p h w", w=WP)
                    src = s1[(img, pc)].rearrange("p (h w) -> p h w", w=W)
                    nc.scalar.activation(
                        dst[:, 1 : 1 + H, 1 : 1 + W], src,
                        mybir.ActivationFunctionType.Sign,
                        bias=c1[:, pc, :], scale=a1[:, pc, :],
                    )
                for pc in range(NPC):
                    s2t = accpool.tile([128, NPIX], F16, name=f"s2_{img}_{pc}",
                                       tag="acc")
                    s2[(img, pc)] = s2t
                    for chunk in range(NCHUNK):
                        ps = _emit_conv(nc, psum_pool, wsb2, sh_tile, pc, chunk)
                        y0 = chunk * CHUNK_ROWS
                        xr = xrpool.tile([128, CHUNK_ROWS, W], F32, name="xr",
                                         tag="xr")
                        nc.sync.dma_start(
                            out=xr,
                            in_=x_d[img, pc * 128 : (pc + 1) * 128,
                                    y0 : y0 + CHUNK_ROWS],
                        )
                        sl = slice(chunk * CHW, (chunk + 1) * CHW)
                        nc.vector.tensor_tensor(
                            out=s2t[:, sl],
                            in0=ps[:],
                            in1=xr.rearrange("p h w -> p (h w)"),
                            op=mybir.AluOpType.add,
                        )
                        nc.vector.bn_stats(
                            out=bnst2[pc][:, img * NCHUNK + chunk, :],
                            in_=s2t[:, sl],
                        )

            # ---- bn2 stats -> AllReduce -> scale/bias ----
            allin2 = singles.tile([128, NPC, 2], F32)
            for pc in range(NPC):
                mv2 = stpool.tile([128, 2], F32, name=f"mv2_{pc}")
                nc.vector.bn_aggr(out=mv2, in_=bnst2[pc])
                nc.vector.tensor_copy(allin2[:, pc, 0:1], mv2[:, 0:1])
                sq2 = stpool.tile([128, 1], F32, name=f"sq2_{pc}")
                nc.vector.tensor_mul(sq2, mv2[:, 0:1], mv2[:, 0:1])
                nc.vector.tensor_tensor(
                    out=allin2[:, pc, 1:2], in0=mv2[:, 1:2], in1=sq2,
                    op=mybir.AluOpType.add,
                )
            cc2_in = dram.tile([128, NPC * 2], F32, name="cc2_in")
            cc2_out = dram.tile([128, NPC * 2], F32, name="cc2_out",
                                addr_space=cc_addr_space)
            nc.gpsimd.dma_start(out=cc2_in, in_=allin2.rearrange("p a b -> p (a b)"))
            nc.gpsimd.collective_compute(
                "AllReduce",
                mybir.AluOpType.add,
                replica_groups=[list(range(ncores))],
                ins=[cc2_in.opt()],
                outs=[cc2_out.opt()],
            )
            allout2 = singles.tile([128, NPC, 2], F32)
            nc.gpsimd.dma_start(out=allout2.rearrange("p a b -> p (a b)"),
                                in_=cc2_out)
            a2, c2 = _stats_to_scale_bias(
                nc, singles, allout2, vecs_sb, eps_tile, 2, 3, "bn2", ncores
            )

            # ---- pass C: scale/bias + clip + store ----
            for img in range(bpc):
                for pc in range(NPC):
                    s2t = s2[(img, pc)]
                    for chunk in range(NCHUNK):
                        sl = slice(chunk * CHW, (chunk + 1) * CHW)
                        oc = ocpool.tile([128, CHUNK_ROWS, W], F32, name="oc",
                                         tag="oc")
                        nc.scalar.activation(
                            oc.rearrange("p h w -> p (h w)"), s2t[:, sl],
                            mybir.ActivationFunctionType.Identity,
                            bias=c2[:, pc, :], scale=a2[:, pc, :],
                        )
                        nc.vector.tensor_scalar(
                            out=oc[:], in0=oc[:], scalar1=1.0, scalar2=-1.0,
                            op0=mybir.AluOpType.min, op1=mybir.AluOpType.max,
                        )
                        y0 = chunk * CHUNK_ROWS
                        nc.sync.dma_start(
                            out=out_d[img, pc * 128 : (pc + 1) * 128,
                                      y0 : y0 + CHUNK_ROWS],
                            in_=oc,
                        )

    nc.compile()
    return nc


_PROGRAM = None


def _get_program():
    global _PROGRAM
    if _PROGRAM is None:
        _PROGRAM = build_program()
    return _PROGRAM


def make_in_maps(x, W1, W2, g1, b1, g2, b2, bpc=BPC, ncores=NCORES):
    w1t = _prep_weight(np.asarray(W1))
    w2t = _prep_weight(np.asarray(W2))
    vecs = _prep_vecs(np.asarray(g1), np.asarray(b1), np.asarray(g2),
                      np.asarray(b2))
    x = np.ascontiguousarray(np.asarray(x, dtype=np.float32))
    return [
        {
            "x": x[core * bpc : (core + 1) * bpc],
            "w1": w1t,
            "w2": w2t,
            "vecs": vecs,
        }
        for core in range(ncores)
    ]


def kernel(x, W1, W2, g1, b1, g2, b2, trace=False):
    nc = _get_program()
    in_maps = make_in_maps(x, W1, W2, g1, b1, g2, b2)
    res = run_bass_kernel_spmd(
        nc, in_maps, core_ids=list(range(NCORES)), trace=trace
    )
    out = np.concatenate([res.results[c]["out"] for c in range(NCORES)], axis=0)
    kernel.last_results = res
    return out


# revision 15
# speedup vs baseline: 1.3531x; 1.3531x over previous
"""Binarized ResNet BasicBlock (2x binarized 3x3 conv + batchnorm + hardtanh,
residual) on 8 Trainium2 NeuronCores, data-parallel over batch.

Math (per reference):
  s1  = conv3x3(sign(x), sign(W1), pad=1)          # integer-valued
  h   = clip(bn1(s1), -1, 1)                       # only sign(h) is consumed
  s2p = conv3x3(sign(h), sign(W2), pad=1) + x
  out = clip(bn2(s2p), -1, 1)

Key points:
  - sign(h) = sign(a1*s1 + c1) per channel (a1 = g1*rsqrt(v1+eps),
    c1 = b1 - m1*a1), so h is never materialized.
  - batchnorm needs global batch stats: each core computes per-channel
    (E[x], E[x^2]) partials over its 4 images; a tiny AllReduce (128x6 f32)
    combines them (equal pixel counts per core, so mean-of-means works).
  - fp8 mode (default): +/-1 activations/weights in fp8e4 are exact; the
    3x3 conv's 27 (channel-chunk, tap) units are packed into 13 DoubleRow
    K=256 matmuls + 1 normal K=128 matmul per output tile. The rhs must be
    a depth-2 AP (pair dim + one run), so each tile reads contiguous
    464-wide runs of the 58-col padded plane and the evacuation strips the
    2-col seam. Pair base addresses must be 2B-aligned and pair strides
    16B-aligned, so shifted copies of the sign plane (made by GPSIMD, which
    is otherwise idle) provide the dx=1 taps (shift +1 col) and a
    row-shifted plane pairs up the leftover dx=2 taps.
  - s1 and s2p stay resident in SBUF as fp16 (integers < 2048: exact; s2p
    adds the fp32 residual, fp16 rounding ~5e-4 relative).
"""

import contextlib

import numpy as np
import ml_dtypes

import concourse.bass as bass
import concourse.tile as tile
from concourse import bacc, mybir
from concourse.bass_utils import run_bass_kernel_spmd
from concourse.replica_groups import maybe_share_collective_output_space

F32 = mybir.dt.float32
F16 = mybir.dt.float16
BF16 = mybir.dt.bfloat16
F8 = mybir.dt.float8e4
F8NP = mybir.dt.np(F8)

NCORES = 8
B, C, H, W = 32, 384, 56, 56
P = C
BPC = B // NCORES         # images per core
NCC = C // 128            # input channel chunks
NPC = P // 128            # output channel chunks
HP = H + 2                # padded rows
WP = W + 2                # padded cols
PADPIX = HP * WP          # 3364
NPIX = H * W              # 3136
CHUNK_ROWS = 8            # output rows per PSUM tile
NCHUNK = H // CHUNK_ROWS  # 7
CHW = CHUNK_ROWS * W      # 448
EPS = 1e-5

MODE = "fp8"              # "bf16" or "fp8"
FP8_SWIL = False          # plain DoubleRow beat SwInterleave on HW
SHIFT_ENGINE = "dma"      # "dma" | "gpsimd" | "vector": shifted-plane copies
CSTRIDE = 3376            # fp8 padded plane stride (16B-aligned)
RUN = CHUNK_ROWS * WP     # 464 <= 512: contiguous rhs run incl. seam
NPLANE = 7                # A0 B0 A1 B1 A2 B2 X2

# fp8 unit schedule: 13 DoubleRow pairs + 1 single cover the 27 (cc, dy, dx)
# conv units. Planes: A-cc at 2cc (shift 0), B-cc at 2cc+1 (shift +1 col,
# provides dx=1 taps at even base addresses), X2 at 6 (A2 shifted one row,
# provides the dy+1 partner for cc2 dx=2 taps).
#  dx01 pair (cc, dy): taps (cc,dy,0)@A-cc, (cc,dy,1)@B-cc;
#    rhs sx[:, 2cc:2cc+2, q:q+RUN], q=(y0+dy)*WP
#  cc01 pair (dy): taps (0,dy,2)@A0, (1,dy,2)@A1;
#    rhs sx[:, 0:3:2, q:q+RUN], q=(y0+dy)*WP+2
#  xp pair: taps (2,0,2)@A2, (2,1,2)@X2; rhs sx[:, 4:7:2, q:q+RUN],
#    q=y0*WP+2
#  single: tap (2,2,2)@A2; rhs sx[:, 4, q:q+RUN], q=(y0+2)*WP+2
FP8_PAIRS = (
    [("dx01", cc, dy) for dy in range(3) for cc in range(3)]
    + [("cc01", None, dy) for dy in range(3)]
    + [("xp", None, None)]
)
NUNIT_FP8 = len(FP8_PAIRS) + 1  # 14


def _prep_weight(w):
    """bf16 mode: [P, C, 3, 3] -> [NCC, 128, 9*NPC*128] bf16 sign; lhsT for
    (cc, off, pc) is wsb[cc][:, (off*NPC+pc)*128 : +128]."""
    ws = np.sign(w.astype(np.float32)).astype(ml_dtypes.bfloat16)
    arr = ws.transpose(1, 2, 3, 0).reshape(C, 9, P)
    arr = arr.reshape(NCC, 128, 9, NPC, 128)
    return np.ascontiguousarray(arr.reshape(NCC, 128, 9 * NPC * 128))


def _fp8_pair_units():
    """(uA, uB) tap indices per FP8_PAIRS entry; each tap is (cc, dy, dx)."""
    out = []
    for kind, cc, dy in FP8_PAIRS:
        if kind == "dx01":
            out.append(((cc, dy, 0), (cc, dy, 1)))
        elif kind == "cc01":
            out.append(((0, dy, 2), (1, dy, 2)))
        else:
            out.append(((2, 0, 2), (2, 1, 2)))
    return out


def _prep_weight_fp8(w):
    """[P, C, 3, 3] -> (pairs [128, 13*NPC*256], single [128, NPC*128]) fp8
    sign values. SwInterleave layout: per pair/pc block of 256, columns are
    [A127, B127, A126, B126, ..., A0, B0] (interleaved, reversed)."""
    ws = np.sign(w.astype(np.float32))
    arr = ws.transpose(1, 2, 3, 0).reshape(NCC, 128, 3, 3, NPC, 128)

    def unit(cc, dy, dx):  # [128 (c), NPC, 128 (m)]
        return arr[cc, :, dy, dx]

    npair = len(FP8_PAIRS)
    if FP8_SWIL:
        wp = np.zeros((128, npair, NPC, 256), np.float32)
        for j, (uA, uB) in enumerate(_fp8_pair_units()):
            wp[:, j, :, 0::2] = unit(*uA)[:, :, ::-1]
            wp[:, j, :, 1::2] = unit(*uB)[:, :, ::-1]
    else:
        wp = np.zeros((128, npair, NPC, 2, 128), np.float32)
        for j, (uA, uB) in enumerate(_fp8_pair_units()):
            wp[:, j, :, 0] = unit(*uA)
            wp[:, j, :, 1] = unit(*uB)
    wsg = unit(2, 2, 2)  # [128, NPC, 128]
    return (
        np.ascontiguousarray(wp.reshape(128, -1)).astype(F8NP),
        np.ascontiguousarray(wsg.reshape(128, -1)).astype(F8NP),
    )


def _prep_vecs(g1, b1, g2, b2):
    """-> [128, NPC, 4] f32: per-partition (p_in) per-chunk (pc) gamma/beta."""
    out = np.empty((128, NPC, 4), np.float32)
    for k, v in enumerate((g1, b1, g2, b2)):
        out[:, :, k] = v.astype(np.float32).reshape(NPC, 128).T
    return out


def _stats_to_scale_bias(nc, singles, allout, vecs_sb, eps_tile, gk, bk, name,
                         ncores):
    """allout [128, NPC, 2] summed (E, E2) over cores -> a, c [128, NPC, 1]."""
    Eg = singles.tile([128, NPC, 1], F32, name=f"{name}_Eg")
    E2g = singles.tile([128, NPC, 1], F32, name=f"{name}_E2g")
    var = singles.tile([128, NPC, 1], F32, name=f"{name}_var")
    tmp = singles.tile([128, NPC, 1], F32, name=f"{name}_tmp")
    sd = singles.tile([128, NPC, 1], F32, name=f"{name}_sd")
    rs = singles.tile([128, NPC, 1], F32, name=f"{name}_rs")
    a = singles.tile([128, NPC, 1], F32, name=f"{name}_a")
    c = singles.tile([128, NPC, 1], F32, name=f"{name}_c")
    nc.scalar.mul(Eg[:], allout[:, :, 0:1], 1.0 / ncores)
    nc.scalar.mul(E2g[:], allout[:, :, 1:2], 1.0 / ncores)
    nc.vector.tensor_mul(tmp[:], Eg[:], Eg[:])
    nc.vector.tensor_tensor(
        out=var[:], in0=E2g[:], in1=tmp[:], op=mybir.AluOpType.subtract
    )
    nc.scalar.activation(
        sd[:], var[:], mybir.ActivationFunctionType.Sqrt, bias=eps_tile[:],
        scale=1.0,
    )
    nc.vector.reciprocal(out=rs[:], in_=sd[:])
    nc.vector.tensor_mul(a[:], rs[:], vecs_sb[:, :, gk : gk + 1])
    nc.vector.tensor_mul(tmp[:], Eg[:], a[:])
    nc.vector.tensor_tensor(
        out=c[:], in0=vecs_sb[:, :, bk : bk + 1], in1=tmp[:],
        op=mybir.AluOpType.subtract,
    )
    return a, c


def _emit_conv_bf16(nc, psum_pool, wsb, sx_tile, pc, chunk):
    """27 accumulated bf16 matmuls -> psum tile [128, CHW]."""
    ps = psum_pool.tile([128, CHW], F32, name="ps", tag="ps")
    y0 = chunk * CHUNK_ROWS
    k = 0
    for cc in range(NCC):
        sx3 = sx_tile[:, cc, :].rearrange("p (h w) -> p h w", w=WP)
        for off in range(9):
            dy, dx = off // 3, off % 3
            lhsT = wsb[cc][:, (off * NPC + pc) * 128 : (off * NPC + pc + 1) * 128]
            rhs = sx3[:, y0 + dy : y0 + dy + CHUNK_ROWS, dx : dx + W]
            nc.tensor.matmul(
                ps[:], lhsT, rhs, start=(k == 0), stop=(k == 9 * NCC - 1)
            )
            k += 1
    return ps


CHUNK_SETS = [range(0, NCHUNK)]


def _emit_conv_fp8(nc, psum_pool, wp_view, ws_view, sx_tile, pc):
    """Weight-stationary fp8 DoubleRow conv for one (img, pc): returns NCHUNK
    psum tiles [128, RUN]; valid output cols = (8, 58) view sliced [:, :56].
    All 7 chunks accumulate in one weight-stationary pass (7 of 8 PSUM
    banks; splitting into half-groups measured slower — the extra DoubleRow
    LDWEIGHTS cost more than the bank stalls they saved)."""
    perf = (mybir.MatmulPerfMode.DoubleRowSwInterleave if FP8_SWIL
            else mybir.MatmulPerfMode.DoubleRow)
    pss = {}
    for cset in CHUNK_SETS:
        for chunk in cset:
            pss[chunk] = psum_pool.tile([128, RUN], F32, name="ps", tag="ps")
        u = 0
        for j, (kind, cc, dy) in enumerate(FP8_PAIRS):
            lhsT = wp_view[:, j, pc]
            for chunk in cset:
                y0 = chunk * CHUNK_ROWS
                if kind == "dx01":
                    q = (y0 + dy) * WP
                    rhs = sx_tile[:, 2 * cc : 2 * cc + 2, q : q + RUN]
                elif kind == "cc01":
                    q = (y0 + dy) * WP + 2
                    rhs = sx_tile[:, 0:3:2, q : q + RUN]
                else:  # xp
                    q = y0 * WP + 2
                    rhs = sx_tile[:, 4:7:2, q : q + RUN]
                nc.tensor.matmul(
                    pss[chunk][:], lhsT, rhs,
                    start=(u == 0), stop=(u == NUNIT_FP8 - 1), perf_mode=perf,
                )
            u += 1
        lhsT = ws_view[:, pc]
        for chunk in cset:
            y0 = chunk * CHUNK_ROWS
            q = (y0 + 2) * WP + 2
            rhs = sx_tile[:, 4, q : q + RUN]
            nc.tensor.matmul(
                pss[chunk][:], lhsT, rhs,
                start=(u == 0), stop=(u == NUNIT_FP8 - 1),
            )
    return [pss[c] for c in range(NCHUNK)]


def build_program(bpc=BPC, ncores=NCORES, mode=MODE, timing_iters=None):
    nc = bacc.Bacc(
        "TRN2",
        target_bir_lowering=False,
        debug=False,
        enable_asserts=True,
        num_devices=ncores,
    )
    x_d = nc.dram_tensor("x", [bpc, C, H, W], F32, kind="ExternalInput").ap()
    if mode == "bf16":
        w1_d = nc.dram_tensor("w1", [NCC, 128, 9 * NPC * 128], BF16,
                              kind="ExternalInput").ap()
        w2_d = nc.dram_tensor("w2", [NCC, 128, 9 * NPC * 128], BF16,
                              kind="ExternalInput").ap()
    else:
        wpair_elems = len(FP8_PAIRS) * NPC * 256
        w1p_d = nc.dram_tensor("w1p", [128, wpair_elems], F8,
                               kind="ExternalInput").ap()
        w1s_d = nc.dram_tensor("w1s", [128, NPC * 128], F8,
                               kind="ExternalInput").ap()
        w2p_d = nc.dram_tensor("w2p", [128, wpair_elems], F8,
                               kind="ExternalInput").ap()
        w2s_d = nc.dram_tensor("w2s", [128, NPC * 128], F8,
                               kind="ExternalInput").ap()
    vecs_d = nc.dram_tensor("vecs", [128, NPC, 4], F32,
                            kind="ExternalInput").ap()
    out_d = nc.dram_tensor("out", [bpc, C, H, W], F32,
                           kind="ExternalOutput").ap()

    with tile.TileContext(nc) as tc:
        with (
            tc.tile_pool(name="weights",
                         bufs=NCC if mode == "bf16" else 2) as wpool,
            tc.tile_pool(name="singles", bufs=1) as singles,
            tc.tile_pool(name="sx", bufs=1) as sxpool,
            tc.tile_pool(name="acc", bufs=3 * bpc) as accpool,
            tc.tile_pool(name="xin", bufs=6) as xinpool,
            tc.tile_pool(name="xr", bufs=6) as xrpool,
            tc.tile_pool(name="oc", bufs=6) as ocpool,
            tc.tile_pool(name="stats", bufs=1) as stpool,
            tc.tile_pool(name="psum", bufs=8, space="PSUM") as psum_pool,
            tc.tile_pool(name="dram", bufs=1, space="DRAM") as dram,
        ):
            # ---- constants (outside the timing loop) ----
            if mode == "bf16":
                # W1/W2 share slots (tag "w"): W2 allocates after conv1
                # releases W1, overlapping the bn1 AllReduce.
                wsb1 = []
                for cc in range(NCC):
                    t1 = wpool.tile([128, 9 * NPC * 128], BF16,
                                    name=f"w1sb{cc}", tag="w")
                    nc.sync.dma_start(out=t1, in_=w1_d[cc])
                    wsb1.append(t1)
            else:
                w1p_sb = wpool.tile([128, len(FP8_PAIRS) * NPC * 256], F8,
                                    name="w1p_sb", tag="wp")
                nc.sync.dma_start(out=w1p_sb, in_=w1p_d)
                w1s_sb = wpool.tile([128, NPC * 128], F8, name="w1s_sb",
                                    tag="ws")
                nc.sync.dma_start(out=w1s_sb, in_=w1s_d)
                w2p_sb = wpool.tile([128, len(FP8_PAIRS) * NPC * 256], F8,
                                    name="w2p_sb", tag="wp")
                nc.sync.dma_start(out=w2p_sb, in_=w2p_d)
                w2s_sb = wpool.tile([128, NPC * 128], F8, name="w2s_sb",
                                    tag="ws")
                nc.sync.dma_start(out=w2s_sb, in_=w2s_d)
                if FP8_SWIL:
                    w1p_v = w1p_sb.rearrange("p (j q m) -> p j q m",
                                             j=len(FP8_PAIRS), q=NPC)
                    w2p_v = w2p_sb.rearrange("p (j q m) -> p j q m",
                                             j=len(FP8_PAIRS), q=NPC)
                else:
                    w1p_v = w1p_sb.rearrange("p (j q i m) -> p j q i m",
                                             j=len(FP8_PAIRS), q=NPC, i=2)
                    w2p_v = w2p_sb.rearrange("p (j q i m) -> p j q i m",
                                             j=len(FP8_PAIRS), q=NPC, i=2)
                w1s_v = w1s_sb.rearrange("p (q m) -> p q m", q=NPC)
                w2s_v = w2s_sb.rearrange("p (q m) -> p q m", q=NPC)
            vecs_sb = singles.tile([128, NPC, 4], F32)
            nc.sync.dma_start(out=vecs_sb, in_=vecs_d)
            eps_tile = singles.tile([128, 1], F32)
            nc.vector.memset(eps_tile, EPS)

            # persistent padded sign planes (borders stay zero forever)
            sxt = []
            for s in range(2):
                if mode == "bf16":
                    t = sxpool.tile([128, NCC, PADPIX], BF16, name=f"sx{s}")
                    nc.gpsimd.memset(t[:], 0.0)
                else:
                    t = sxpool.tile([128, NPLANE, CSTRIDE], F8, name=f"sx{s}")
                    # split the clear across idle engines (serial gpsimd
                    # memset of 21KB/partition would gate the first conv)
                    for pl in range(NPLANE):
                        eng = (nc.vector, nc.gpsimd, nc.scalar)[pl % 3]
                        if eng is nc.scalar:
                            eng.memzero(t[:, pl, :])
                        else:
                            eng.memset(t[:, pl, :], 0.0)
                sxt.append(t)

            bnst1 = [
                stpool.tile([128, bpc * NCHUNK, 6], F32, name=f"bnst1_{pc}")
                for pc in range(NPC)
            ]
            bnst2 = [
                stpool.tile([128, bpc * NCHUNK, 6], F32, name=f"bnst2_{pc}")
                for pc in range(NPC)
            ]

            cc_addr_space = (
                "Local" if timing_iters is not None
                else maybe_share_collective_output_space(
                    "AllReduce", [list(range(ncores))]
                )
            )

            def do_allreduce(cin, cout):
                if timing_iters is None:
                    nc.gpsimd.collective_compute(
                        "AllReduce",
                        mybir.AluOpType.add,
                        replica_groups=[list(range(ncores))],
                        ins=[cin.opt()],
                        outs=[cout.opt()],
                    )
                else:
                    nc.gpsimd.dma_start(out=cout, in_=cin)

            def make_shift_copies(sx_tile):
                """B planes = A planes shifted +1 col; X2 = A2 shifted one
                row. Pad regions supply the zeros."""
                pairs = [
                    (sx_tile[:, 1:6:2, 0 : CSTRIDE - 1],
                     sx_tile[:, 0:5:2, 1:CSTRIDE]),
                    (sx_tile[:, 6, 0 : CSTRIDE - WP],
                     sx_tile[:, 4, WP:CSTRIDE]),
                ]
                for o, i in pairs:
                    if SHIFT_ENGINE == "dma":
                        nc.sync.dma_start(out=o, in_=i)
                    elif SHIFT_ENGINE == "vector":
                        nc.vector.tensor_copy(o, i)
                    else:
                        nc.gpsimd.tensor_copy(out=o, in_=i)

            loop_cm = (tc.For_i(0, timing_iters, 1) if timing_iters
                       else contextlib.nullcontext())
            with loop_cm:
                # ---- pass A: conv1, stats, s1 resident in fp16 ----
                s1 = {}
                s2 = {}
                for img in range(bpc):
                    sx_tile = sxt[img % 2]
                    for cc in range(NCC):
                        if mode == "bf16":
                            dst = sx_tile[:, cc, :].rearrange(
                                "p (h w) -> p h w", w=WP)
                        else:
                            dst = sx_tile[:, 2 * cc, :PADPIX].rearrange(
                                "p (h w) -> p h w", w=WP)
                        for chunk in range(NCHUNK):
                            y0 = chunk * CHUNK_ROWS
                            xin = xinpool.tile([128, CHUNK_ROWS, W], F32,
                                               name="xin", tag="xin")
                            nc.sync.dma_start(
                                out=xin,
                                in_=x_d[img, cc * 128 : (cc + 1) * 128,
                                        y0 : y0 + CHUNK_ROWS],
                            )
                            nc.scalar.activation(
                                dst[:, 1 + y0 : 1 + y0 + CHUNK_ROWS,
                                    1 : 1 + W],
                                xin, mybir.ActivationFunctionType.Sign,
                            )
                    if mode == "fp8":
                        make_shift_copies(sx_tile)
                    for pc in range(NPC):
                        s1t = accpool.tile([128, NPIX], F16,
                                           name=f"s1_{img}_{pc}", tag="acc")
                        s1[(img, pc)] = s1t
                        if mode == "bf16":
                            for chunk in range(NCHUNK):
                                ps = _emit_conv_bf16(nc, psum_pool, wsb1,
                                                     sx_tile, pc, chunk)
                                sl = slice(chunk * CHW, (chunk + 1) * CHW)
                                nc.scalar.copy(s1t[:, sl], ps[:])
                                nc.vector.bn_stats(
                                    out=bnst1[pc][:, img * NCHUNK + chunk, :],
                                    in_=ps[:],
                                )
                        else:
                            pss = _emit_conv_fp8(nc, psum_pool, w1p_v, w1s_v,
                                                 sx_tile, pc)
                            for chunk in range(NCHUNK):
                                ps_v = (pss[chunk]
                                        .rearrange("p (r c) -> p r c", c=WP)
                                        [:, :, 0:W])
                                sl = slice(chunk * CHW, (chunk + 1) * CHW)
                                s1_v = s1t[:, sl].rearrange(
                                    "p (r c) -> p r c", c=W)
                                nc.scalar.copy(s1_v, ps_v)
                                nc.vector.bn_stats(
                                    out=bnst1[pc][:, img * NCHUNK + chunk, :],
                                    in_=s1t[:, sl],
                                )

                # ---- bn1 stats -> AllReduce -> thresholds ----
                allin1 = singles.tile([128, NPC, 2], F32)
                for pc in range(NPC):
                    mv = stpool.tile([128, 2], F32, name=f"mv1_{pc}")
                    nc.vector.bn_aggr(out=mv, in_=bnst1[pc])
                    nc.vector.tensor_copy(allin1[:, pc, 0:1], mv[:, 0:1])
                    sq = stpool.tile([128, 1], F32, name=f"sq1_{pc}")
                    nc.vector.tensor_mul(sq, mv[:, 0:1], mv[:, 0:1])
                    nc.vector.tensor_tensor(
                        out=allin1[:, pc, 1:2], in0=mv[:, 1:2], in1=sq,
                        op=mybir.AluOpType.add,
                    )
                cc1_in = dram.tile([128, NPC * 2], F32, name="cc1_in")
                cc1_out = dram.tile([128, NPC * 2], F32, name="cc1_out",
                                    addr_space=cc_addr_space)
                nc.gpsimd.dma_start(
                    out=cc1_in, in_=allin1.rearrange("p a b -> p (a b)"))
                do_allreduce(cc1_in, cc1_out)
                allout1 = singles.tile([128, NPC, 2], F32)
                nc.gpsimd.dma_start(
                    out=allout1.rearrange("p a b -> p (a b)"), in_=cc1_out)
                a1, c1 = _stats_to_scale_bias(
                    nc, singles, allout1, vecs_sb, eps_tile, 0, 1, "bn1",
                    ncores,
                )

                # ---- W2 into the shared weight slots (bf16 mode) ----
                if mode == "bf16":
                    wsb2 = []
                    for cc in range(NCC):
                        t2 = wpool.tile([128, 9 * NPC * 128], BF16,
                                        name=f"w2sb{cc}", tag="w")
                        nc.sync.dma_start(out=t2, in_=w2_d[cc])
                        wsb2.append(t2)

                # ---- pass B: sign threshold, conv2 + residual, stats ----
                for img in range(bpc):
                    sh_tile = sxt[img % 2]
                    for pc in range(NPC):
                        srcv = s1[(img, pc)].rearrange("p (h w) -> p h w", w=W)
                        if mode == "bf16":
                            dst = sh_tile[:, pc, :].rearrange(
                                "p (h w) -> p h w", w=WP)
                        else:
                            dst = sh_tile[:, 2 * pc, :PADPIX].rearrange(
                                "p (h w) -> p h w", w=WP)
                        nc.scalar.activation(
                            dst[:, 1 : 1 + H, 1 : 1 + W], srcv,
                            mybir.ActivationFunctionType.Sign,
                            bias=c1[:, pc, :], scale=a1[:, pc, :],
                        )
                    if mode == "fp8":
                        make_shift_copies(sh_tile)
                    for pc in range(NPC):
                        s2t = accpool.tile([128, NPIX], F16,
                                           name=f"s2_{img}_{pc}", tag="acc")
                        s2[(img, pc)] = s2t
                        if mode == "bf16":
                            pss = [_emit_conv_bf16(nc, psum_pool, wsb2,
                                                   sh_tile, pc, chunk)
                                   for chunk in range(NCHUNK)]
                        else:
                            pss = _emit_conv_fp8(nc, psum_pool, w2p_v, w2s_v,
                                                 sh_tile, pc)
                        for chunk in range(NCHUNK):
                            y0 = chunk * CHUNK_ROWS
                            xr = xrpool.tile([128, CHUNK_ROWS, W], F32,
                                             name="xr", tag="xr")
                            nc.sync.dma_start(
                                out=xr,
                                in_=x_d[img, pc * 128 : (pc + 1) * 128,
                                        y0 : y0 + CHUNK_ROWS],
                            )
                            sl = slice(chunk * CHW, (chunk + 1) * CHW)
                            if mode == "bf16":
                                ps_in = pss[chunk][:]
                                xr_in = xr.rearrange("p h w -> p (h w)")
                                out_ap = s2t[:, sl]
                            else:
                                ps_in = (pss[chunk]
                                         .rearrange("p (r c) -> p r c", c=WP)
                                         [:, :, 0:W])
                                xr_in = xr[:]
                                out_ap = s2t[:, sl].rearrange(
                                    "p (r c) -> p r c", c=W)
                            nc.vector.tensor_tensor(
                                out=out_ap, in0=ps_in, in1=xr_in,
                                op=mybir.AluOpType.add,
                            )
                            nc.vector.bn_stats(
                                out=bnst2[pc][:, img * NCHUNK + chunk, :],
                                in_=s2t[:, sl],
                            )

                # ---- bn2 stats -> AllReduce -> scale/bias ----
                allin2 = singles.tile([128, NPC, 2], F32)
                for pc in range(NPC):
                    mv2 = stpool.tile([128, 2], F32, name=f"mv2_{pc}")
                    nc.vector.bn_aggr(out=mv2, in_=bnst2[pc])
                    nc.vector.tensor_copy(allin2[:, pc, 0:1], mv2[:, 0:1])
                    sq2 = stpool.tile([128, 1], F32, name=f"sq2_{pc}")
                    nc.vector.tensor_mul(sq2, mv2[:, 0:1], mv2[:, 0:1])
                    nc.vector.tensor_tensor(
                        out=allin2[:, pc, 1:2], in0=mv2[:, 1:2], in1=sq2,
                        op=mybir.AluOpType.add,
                    )
                cc2_in = dram.tile([128, NPC * 2], F32, name="cc2_in")
                cc2_out = dram.tile([128, NPC * 2], F32, name="cc2_out",
                                    addr_space=cc_addr_space)
                nc.gpsimd.dma_start(
                    out=cc2_in, in_=allin2.rearrange("p a b -> p (a b)"))
                do_allreduce(cc2_in, cc2_out)
                allout2 = singles.tile([128, NPC, 2], F32)
                nc.gpsimd.dma_start(
                    out=allout2.rearrange("p a b -> p (a b)"), in_=cc2_out)
                a2, c2 = _stats_to_scale_bias(
                    nc, singles, allout2, vecs_sb, eps_tile, 2, 3, "bn2",
                    ncores,
                )

                # ---- pass C: scale/bias + clip + store ----
                for img in range(bpc):
                    for pc in range(NPC):
                        s2t = s2[(img, pc)]
                        for chunk in range(NCHUNK):
                            sl = slice(chunk * CHW, (chunk + 1) * CHW)
                            oc = ocpool.tile([128, CHUNK_ROWS, W], F32,
                                             name="oc", tag="oc")
                            ocf = oc.rearrange("p h w -> p (h w)")
                            if chunk % 2 == 0:
                                nc.scalar.activation(
                                    ocf, s2t[:, sl],
                                    mybir.ActivationFunctionType.Identity,
                                    bias=c2[:, pc, :], scale=a2[:, pc, :],
                                )
                                nc.vector.tensor_scalar(
                                    out=oc[:], in0=oc[:], scalar1=1.0,
                                    scalar2=-1.0, op0=mybir.AluOpType.min,
                                    op1=mybir.AluOpType.max,
                                )
                            else:
                                nc.vector.tensor_scalar(
                                    out=ocf, in0=s2t[:, sl],
                                    scalar1=a2[:, pc, :],
                                    scalar2=c2[:, pc, :],
                                    op0=mybir.AluOpType.mult,
                                    op1=mybir.AluOpType.add,
                                )
                                nc.gpsimd.tensor_scalar(
                                    out=oc[:], in0=oc[:], scalar1=1.0,
                                    scalar2=-1.0, op0=mybir.AluOpType.min,
                                    op1=mybir.AluOpType.max,
                                )
                            y0 = chunk * CHUNK_ROWS
                            nc.sync.dma_start(
                                out=out_d[img, pc * 128 : (pc + 1) * 128,
                                          y0 : y0 + CHUNK_ROWS],
                                in_=oc,
                            )

    nc.compile()
    return nc


_PROGRAM = None


def _get_program():
    global _PROGRAM
    if _PROGRAM is None:
        _PROGRAM = build_program()
    return _PROGRAM


def make_in_maps(x, W1, W2, g1, b1, g2, b2, bpc=BPC, ncores=NCORES,
                 mode=MODE):
    vecs = _prep_vecs(np.asarray(g1), np.asarray(b1), np.asarray(g2),
                      np.asarray(b2))
    x = np.ascontiguousarray(np.asarray(x, dtype=np.float32))
    if mode == "bf16":
        wmap = {"w1": _prep_weight(np.asarray(W1)),
                "w2": _prep_weight(np.asarray(W2))}
    else:
        w1p, w1s = _prep_weight_fp8(np.asarray(W1))
        w2p, w2s = _prep_weight_fp8(np.asarray(W2))
        wmap = {"w1p": w1p, "w1s": w1s, "w2p": w2p, "w2s": w2s}
    return [
        {"x": x[core * bpc : (core + 1) * bpc], "vecs": vecs, **wmap}
        for core in range(ncores)
    ]


def kernel(x, W1, W2, g1, b1, g2, b2, trace=False):
    nc = _get_program()
    in_maps = make_in_maps(x, W1, W2, g1, b1, g2, b2)
    res = run_bass_kernel_spmd(
        nc, in_maps, core_ids=list(range(NCORES)), trace=trace
    )
    out = np.concatenate([res.results[c]["out"] for c in range(NCORES)], axis=0)
    kernel.last_results = res
    return out
